# revision 7
# baseline (speedup 1.0000x reference)
"""Trainium2 Bass kernel for nn_LinearInFieldChargesBlock (e3nn fully-connected
tensor product, forward only).

Math (per node n):
  out0[w] = 0.01*(C000 * sum_{u,v} x0[u] y0[v] w000[u,v,w]
                 + C110 * sum_{u,v,i} x1[u,i] y1[v,i] w110[u,v,w])
  out1[w,k] = 0.01*(C011 * sum_{u,v} x0[u] y1[v,k] w011[u,v,w]
                 + C101 * sum_{u,v} x1[u,k] y0[v] w101[u,v,w])
  out = concat([out0, out1.reshape(-1)]) with column 0 zeroed.

Kernel formulation:
  The bilinear form is decomposed into 160 rank-1 products q[f,n] =
  p[f,n] * yb[f,n], where p = W^T x (stage-1 PE matmuls contracting the
  512 node features) and yb is the per-node y value selected for each
  product row (one PE broadcast matmul from the 16 pot features).
  out[o,n] = sum_f R[f,o] q[f,n] (0/1 reduce matmuls, col 0 zeroed).

  The 160 product rows are split 128 ("A", psum tile pa) + 32 ("B", pb at
  partitions 32..63).  Row order is chosen so the B rows' y factors equal
  the factors of A rows 32..63 -- the B multiply reuses those rows of the
  A broadcast tile, so only ONE broadcast matmul is needed.

  PE work is packed with tile_position concurrency into ~3 x 512-cycle
  rounds per 512-node sub-batch:
    [c0 | c1 | c2]  stage-1 chunks at array col-groups {0,1},{2},{3}
    [pb | R_A(prev)] col-groups {1},{0}
    [R_B(prev) | Yb_A]  cells (row1,col0) vs (row0, cols0-3)
  The reduce matmuls of sub-batch b are emitted during sub-batch b+1 so
  the in-order PE queue never stalls on the DVE/ACT q-chain.

  All data is staged bf16 host-side in transposed layout (features on
  partitions), halving HBM traffic and removing all on-chip transposes.

Sharding: pure data-parallel across 8 cores along the node axis; the tiny
path-weight matrices are replicated.
"""

import sys

import numpy as np

try:
    import concourse  # noqa: F401
except ImportError:
    sys.path.insert(0, "/opt/trn_rl_repo")

N_NODES = 400000
N_CORES = 8
BATCH = 512            # nodes per PSUM sub-batch
SUB_PER_CHUNK = 7      # sub-batches per DMA chunk
CHUNK = BATCH * SUB_PER_CHUNK  # 3584 nodes per DMA chunk
N_CHUNKS = 14
PER_CORE = CHUNK * N_CHUNKS    # 50176 >= ceil(400000/8)
PADDED = PER_CORE * N_CORES

_S = 0.01
_CS000 = _S / 32.0
_CS110 = _S / (32.0 * np.sqrt(3.0))
_CS011 = _S / 32.0
_CS101 = _S / 32.0


def _bf16():
    import ml_dtypes

    return np.dtype(ml_dtypes.bfloat16)


def _build_mats(w000, w011, w101, w110):
    """Build the stage-1 weight blocks and the selector matrices.

    Product-row layout (f = row index):
      pa rows  0..15 : t011 copy k=0  (chunk0)  factor y1[v,0]   f=4v+w
      pa rows 16..31 : t011 copy k=1  (chunk0)  factor y1[v,1]
      pa rows 32..47 : t011 copy k=2  (chunk0)  factor y1[v,2]
      pa rows 48..63 : t000           (chunk0)  factor y0[v]
      pa rows 64..79 : t110 i=0       (chunk1)  factor y1[v,0]
      pa rows 80..95 : t101 k=0       (chunk1)  factor y0[v]
      pa rows 96..111: t110 i=1       (chunk2)  factor y1[v,1]
      pa rows112..127: t101 k=1       (chunk2)  factor y0[v]
      pb parts 32..47: t110 i=2       (chunk3)  factor y1[v,2] == yba rows 32..47
      pb parts 48..63: t101 k=2       (chunk3)  factor y0[v]   == yba rows 48..63

    y feature order (natural pot_feat layout): b=v for y0[v]; b=4+3v+i for y1[v,i].
    out column order: o=w for out0[w]; o=4+3w+k for out1[w,k]. Column 0 zeroed.
    """
    bf16 = _bf16()
    WA0 = np.zeros((128, 64), np.float32)   # chunk0 -> pa rows 0..63
    W1X = np.zeros((128, 32), np.float32)   # chunks 1..3 -> 32 rows each
    BA = np.zeros((16, 128), np.float32)
    RA = np.zeros((128, 16), np.float32)
    RB = np.zeros((32, 16), np.float32)
    for v in range(4):
        for w in range(4):
            f = 4 * v + w
            # pa blocks 0..2: t011 copies k=0,1,2
            for k in range(3):
                WA0[:, 16 * k + f] = _CS011 * w011[:, v, w]
                BA[4 + 3 * v + k, 16 * k + f] = 1.0
                RA[16 * k + f, 4 + 3 * w + k] = 1.0
            # pa block 3: t000
            WA0[:, 48 + f] = _CS000 * w000[:, v, w]
            BA[v, 48 + f] = 1.0
            if w > 0:
                RA[48 + f, w] = 1.0
            # shared chunk block (chunks 1..3 with i = chunk-1):
            #   rows 0..15: t110[i], rows 16..31: t101[k=i]
            W1X[:, f] = _CS110 * w110[:, v, w]
            W1X[:, 16 + f] = _CS101 * w101[:, v, w]
            # chunk1 (i=0) -> pa rows 64..95
            BA[4 + 3 * v + 0, 64 + f] = 1.0
            if w > 0:
                RA[64 + f, w] = 1.0
            BA[v, 80 + f] = 1.0
            RA[80 + f, 4 + 3 * w + 0] = 1.0
            # chunk2 (i=1) -> pa rows 96..127
            BA[4 + 3 * v + 1, 96 + f] = 1.0
            if w > 0:
                RA[96 + f, w] = 1.0
            BA[v, 112 + f] = 1.0
            RA[112 + f, 4 + 3 * w + 1] = 1.0
            # chunk3 (i=2) -> pb; y factors match yba rows 32..63
            if w > 0:
                RB[f, w] = 1.0
            RB[16 + f, 4 + 3 * w + 2] = 1.0
    return (
        WA0.astype(bf16),
        W1X.astype(bf16),
        BA.astype(bf16),
        RA.astype(bf16),
        RB.astype(bf16),
    )


def _pack_inputs(node_feat, pot_feat):
    """Transpose + pad + bf16-cast the node data. Returns (xT, yT) with
    xT [128, 4, PADDED]: chunk 0 = x0 features, chunks 1..3 = x1[:, :, i-1].
    yT [16, PADDED]."""
    bf16 = _bf16()
    n = node_feat.shape[0]
    xT = np.zeros((128, 4, PADDED), dtype=bf16)
    xT[:, 0, :n] = np.asarray(node_feat[:, :128].T, dtype=bf16)
    x1 = node_feat[:, 128:].reshape(n, 128, 3)
    for c in range(3):
        xT[:, 1 + c, :n] = np.asarray(x1[:, :, c].T, dtype=bf16)
    yT = np.zeros((16, PADDED), dtype=bf16)
    yT[:, :n] = np.asarray(pot_feat.T, dtype=bf16)
    return xT, yT


def build_in_maps(node_feat, pot_feat, w000, w011, w101, w110):
    node_feat = np.asarray(node_feat, dtype=np.float32)
    pot_feat = np.asarray(pot_feat, dtype=np.float32)
    WA0, W1X, BA, RA, RB = _build_mats(
        np.asarray(w000, np.float32),
        np.asarray(w011, np.float32),
        np.asarray(w101, np.float32),
        np.asarray(w110, np.float32),
    )
    xT, yT = _pack_inputs(node_feat, pot_feat)
    in_maps = []
    for i in range(N_CORES):
        sl = slice(i * PER_CORE, (i + 1) * PER_CORE)
        in_maps.append(
            {
                "xt": np.ascontiguousarray(xT[:, :, sl]),
                "yt": np.ascontiguousarray(yT[:, sl]),
                "wa0": WA0,
                "w1x": W1X,
                "ba": BA,
                "ra": RA,
                "rb": RB,
            }
        )
    return in_maps


_CACHE = {}


def build_kernel(n_nodes=PER_CORE):
    """Build + compile the per-core Bass program (n_nodes multiple of CHUNK)."""
    if n_nodes in _CACHE:
        return _CACHE[n_nodes]

    import concourse.bacc as bacc
    import concourse.tile as tile
    from concourse import mybir

    f32 = mybir.dt.float32
    bf = mybir.dt.bfloat16

    assert n_nodes % CHUNK == 0
    n_chunks = n_nodes // CHUNK
    n_batches = n_chunks * SUB_PER_CHUNK

    nc = bacc.Bacc(None, target_bir_lowering=False)
    xtd = nc.dram_tensor("xt", [128, 4, n_nodes], bf, kind="ExternalInput")
    ytd = nc.dram_tensor("yt", [16, n_nodes], bf, kind="ExternalInput")
    wa0d = nc.dram_tensor("wa0", [128, 64], bf, kind="ExternalInput")
    w1xd = nc.dram_tensor("w1x", [128, 32], bf, kind="ExternalInput")
    bad = nc.dram_tensor("ba", [16, 128], bf, kind="ExternalInput")
    rad = nc.dram_tensor("ra", [128, 16], bf, kind="ExternalInput")
    rbd = nc.dram_tensor("rb", [32, 16], bf, kind="ExternalInput")
    outd = nc.dram_tensor("out", [16, n_nodes], bf, kind="ExternalOutput")

    with tile.TileContext(nc) as tc:
        with (
            tc.tile_pool(name="consts", bufs=1) as consts,
            tc.tile_pool(name="xin", bufs=3) as xin,
            tc.tile_pool(name="yin", bufs=3) as yin,
            tc.tile_pool(name="ost", bufs=3) as ostp,
            tc.tile_pool(name="ybs", bufs=3) as ybsp,
            tc.tile_pool(name="pas", bufs=3) as pasp,
            tc.tile_pool(name="qa", bufs=3) as qap,
            tc.tile_pool(name="qb", bufs=3) as qbp,
            tc.tile_pool(name="pa", bufs=2, space="PSUM") as pap,
            tc.tile_pool(name="pb", bufs=2, space="PSUM") as pbp,
            tc.tile_pool(name="yba", bufs=1, space="PSUM") as ybap,
            tc.tile_pool(name="otq", bufs=3, space="PSUM") as otqp,
        ):
            wa0 = consts.tile([128, 64], bf, tag="wa0")
            nc.sync.dma_start(out=wa0[:], in_=wa0d[:])
            w1x = consts.tile([128, 32], bf, tag="w1x")
            nc.sync.dma_start(out=w1x[:], in_=w1xd[:])
            ba = consts.tile([16, 128], bf, tag="ba")
            nc.sync.dma_start(out=ba[:], in_=bad[:])
            ra = consts.tile([128, 16], bf, tag="ra")
            nc.sync.dma_start(out=ra[:], in_=rad[:])
            rbt = consts.tile([64, 16], bf, tag="rb")
            nc.sync.dma_start(out=rbt[32:64, :], in_=rbd[:])

            X = Y = ost = None
            pending = []  # reduce states, drained at pipeline distance 2

            def emit_reduce(state):
                qa_, qb_, ost_, m0_, last_, j0_ = state
                otq = otqp.tile([16, BATCH], f32, tag="otq")
                nc.tensor.matmul(otq[:], ra[:], qa_[:], start=True, stop=False)
                nc.tensor.matmul(
                    otq[:], rbt[32:64, :], qb_[32:64, :], start=False, stop=True
                )
                nc.vector.tensor_copy(ost_[:, m0_ : m0_ + BATCH], otq[:])
                if last_:
                    # out-DMA on the ACT HWDGE ring: the Sync ring carries the
                    # input prefetch and must never head-of-line block on a
                    # DMA that waits for compute.
                    nc.scalar.dma_start(
                        out=outd[:, j0_ : j0_ + CHUNK], in_=ost_[:]
                    )

            for g in range(n_batches):
                ch, sb = divmod(g, SUB_PER_CHUNK)
                j0 = ch * CHUNK
                if sb == 0:
                    X = xin.tile([128, 4 * CHUNK], bf, tag="x")
                    nc.sync.dma_start(
                        out=X[:].rearrange("p (c m) -> p c m", c=4),
                        in_=xtd[:, :, j0 : j0 + CHUNK],
                    )
                    Y = yin.tile([16, CHUNK], bf, tag="y")
                    nc.sync.dma_start(out=Y[:], in_=ytd[:, j0 : j0 + CHUNK])
                    ost = ostp.tile([16, CHUNK], bf, tag="ost")

                m0 = sb * BATCH
                # stage 1: contract the 512 node features.
                # PE slot structure per batch (disjoint array cells overlap):
                #   [c0|c1|c2] -> [pb|R_A(g-2)] -> [R_B(g-2)|Yb_A(g)]
                pa = pap.tile([128, BATCH], f32, tag="pa")
                nc.tensor.matmul(
                    pa[0:64, :],
                    wa0[:],
                    X[:, 0 * CHUNK + m0 : 0 * CHUNK + m0 + BATCH],
                    start=True,
                    stop=True,
                )
                nc.tensor.matmul(
                    pa[64:96, :],
                    w1x[:],
                    X[:, 1 * CHUNK + m0 : 1 * CHUNK + m0 + BATCH],
                    start=True,
                    stop=True,
                )
                nc.tensor.matmul(
                    pa[96:128, :],
                    w1x[:],
                    X[:, 2 * CHUNK + m0 : 2 * CHUNK + m0 + BATCH],
                    start=True,
                    stop=True,
                    tile_position=(0, 96),
                )
                pb = pbp.tile([64, BATCH], f32, tag="pb")
                nc.tensor.matmul(
                    pb[32:64, :],
                    w1x[:],
                    X[:, 3 * CHUNK + m0 : 3 * CHUNK + m0 + BATCH],
                    start=True,
                    stop=True,
                )
                pas = pasp.tile([128, BATCH], bf, tag="pas")
                nc.scalar.copy(out=pas[:], in_=pa[:])
                # reduce of batch g-2 (q-chain has 2 batches of slack, so the
                # PE never stalls waiting on scalar/gpsimd)
                if len(pending) >= 2:
                    emit_reduce(pending.pop(0))
                # broadcast y into the 128 A-rows (selector matmul)
                yba = ybap.tile([128, BATCH], f32, tag="yba")
                nc.tensor.matmul(
                    yba[:], ba[:], Y[:, m0 : m0 + BATCH], start=True, stop=True
                )
                ybs = ybsp.tile([128, BATCH], bf, tag="ybs")
                nc.scalar.copy(out=ybs[:], in_=yba[:])
                # q = p * yb (qa on gpsimd frees the DVE for qb+output)
                qa = qap.tile([128, BATCH], bf, tag="qa")
                nc.gpsimd.tensor_mul(qa[:], pas[:], ybs[:])
                qb = qbp.tile([64, BATCH], bf, tag="qb")
                nc.vector.tensor_mul(qb[32:64, :], pb[32:64, :], ybs[32:64, :])
                pending.append((qa, qb, ost, m0, sb == SUB_PER_CHUNK - 1, j0))
            for state in pending:
                emit_reduce(state)
    nc.compile()
    _CACHE[n_nodes] = nc
    return nc


def kernel(node_feat, pot_feat, w000, w011, w101, w110, **extra_kwargs):
    from concourse.bass_utils import run_bass_kernel_spmd

    n = np.asarray(node_feat).shape[0]
    in_maps = build_in_maps(node_feat, pot_feat, w000, w011, w101, w110)
    nc = build_kernel(PER_CORE)
    res = run_bass_kernel_spmd(nc, in_maps, core_ids=list(range(N_CORES)))
    outT = np.concatenate(
        [np.asarray(res.results[i]["out"]) for i in range(N_CORES)], axis=1
    )
    out = outT[:, :n].T.astype(np.float32)
    return np.ascontiguousarray(out)


# revision 24
# speedup vs baseline: 1.0964x; 1.0964x over previous
"""Trainium2 Bass kernel for nn_LinearInFieldChargesBlock (e3nn fully-connected
tensor product, forward only).

Math (per node n):
  out0[w] = 0.01*(C000 * sum_{u,v} x0[u] y0[v] w000[u,v,w]
                 + C110 * sum_{u,v,i} x1[u,i] y1[v,i] w110[u,v,w])
  out1[w,k] = 0.01*(C011 * sum_{u,v} x0[u] y1[v,k] w011[u,v,w]
                 + C101 * sum_{u,v} x1[u,k] y0[v] w101[u,v,w])
  out = concat([out0, out1.reshape(-1)]) with column 0 zeroed.

Kernel formulation:
  The bilinear form is decomposed into 160 rank-1 products q[f,n] =
  p[f,n] * yb[f,n], where p = W^T x (stage-1 PE matmuls contracting the
  512 node features, path constants folded into W) and yb[f,n] is the per-
  node y value of product f.  out[o,n] = sum_f R[f,o] q[f,n] (0/1 reduce
  matmuls, col 0 zeroed).

  The 160 product rows are split 128 ("A", psum tile pa) + 32 ("B", pb at
  partitions 32..63).  The broadcast tile yb is built once per DMA chunk
  with SBUF->SBUF gather DMAs (partition_broadcast access patterns), so
  the PE only runs 6 matmuls per 512-node sub-batch, packed into 3
  serial array slots via tile_position concurrency:
      [c0|c1|c2|pb]  ->  R_A(g-2)  ->  R_B(g-2)
  The reduce matmuls run at pipeline distance 2 so the in-order PE queue
  never waits on the scalar/DVE/gpsimd q-chain.

  All node data is staged bf16 host-side in transposed layout (features
  on partitions), halving HBM traffic and removing on-chip transposes.

Sharding: pure data-parallel across 8 cores along the node axis; the tiny
path-weight matrices are replicated.
"""

import sys

import numpy as np

try:
    import concourse  # noqa: F401
except ImportError:
    sys.path.insert(0, "/opt/trn_rl_repo")

N_NODES = 400000
N_CORES = 8
BATCH = 512            # nodes per PSUM sub-batch
SUB_PER_CHUNK = 7      # sub-batches per DMA chunk
CHUNK = BATCH * SUB_PER_CHUNK  # 3584 nodes per DMA chunk
N_CHUNKS = 14
PER_CORE = CHUNK * N_CHUNKS    # 50176 >= ceil(400000/8)
PADDED = PER_CORE * N_CORES

_S = 0.01
_CS000 = _S / 32.0
_CS110 = _S / (32.0 * np.sqrt(3.0))
_CS011 = _S / 32.0
_CS101 = _S / 32.0


def _bf16():
    import ml_dtypes

    return np.dtype(ml_dtypes.bfloat16)


def _build_mats(w000, w011, w101, w110):
    """Build the stage-1 weight blocks and the reduce matrices.

    Product-row layout, within-block index r = 4*v + w (v matches the
    gather-DMA partition stride, w is the 0-stride broadcast dim):
      pa rows  0..15 : t011 copy k=0  (chunk0)  factor y1[v,0]
      pa rows 16..31 : t011 copy k=1  (chunk0)  factor y1[v,1]
      pa rows 32..47 : t011 copy k=2  (chunk0)  factor y1[v,2]
      pa rows 48..63 : t000           (chunk0)  factor y0[v]
      pa rows 64..79 : t110 i=0       (chunk1)  factor y1[v,0]
      pa rows 80..95 : t101 k=0       (chunk1)  factor y0[v]
      pa rows 96..111: t110 i=1       (chunk2)  factor y1[v,1]
      pa rows112..127: t101 k=1       (chunk2)  factor y0[v]
      pb parts 32..47: t110 i=2       (chunk3)  factor y1[v,2]
      pb parts 48..63: t101 k=2       (chunk3)  factor y0[v]

    y feature order (natural pot_feat layout): b=v for y0[v]; b=4+3v+i
    for y1[v,i].  out columns: o=w for out0[w]; o=4+3w+k for out1[w,k];
    column 0 zeroed via R.
    """
    bf16 = _bf16()
    WA0 = np.zeros((128, 64), np.float32)   # chunk0 -> pa rows 0..63
    W1X = np.zeros((128, 32), np.float32)   # chunks 1..3 -> 32 rows each
    BA = np.zeros((16, 128), np.float32)
    RA = np.zeros((128, 16), np.float32)
    RB = np.zeros((32, 16), np.float32)
    for v in range(4):
        for w in range(4):
            r = 4 * v + w
            # pa blocks 0..2: t011 copies k=0,1,2
            for k in range(3):
                WA0[:, 16 * k + r] = _CS011 * w011[:, v, w]
                BA[4 + 3 * v + k, 16 * k + r] = 1.0
                RA[16 * k + r, 4 + 3 * w + k] = 1.0
            # pa block 3: t000
            WA0[:, 48 + r] = _CS000 * w000[:, v, w]
            BA[v, 48 + r] = 1.0
            if w > 0:
                RA[48 + r, w] = 1.0
            # shared chunk block (chunks 1..3 with i = chunk-1):
            #   rows 0..15: t110[i], rows 16..31: t101[k=i]
            W1X[:, r] = _CS110 * w110[:, v, w]
            W1X[:, 16 + r] = _CS101 * w101[:, v, w]
            # chunk1 (i=0) -> pa rows 64..95
            BA[4 + 3 * v + 0, 64 + r] = 1.0
            if w > 0:
                RA[64 + r, w] = 1.0
            BA[v, 80 + r] = 1.0
            RA[80 + r, 4 + 3 * w + 0] = 1.0
            # chunk2 (i=1) -> pa rows 96..111
            BA[4 + 3 * v + 1, 96 + r] = 1.0
            if w > 0:
                RA[96 + r, w] = 1.0
            BA[v, 112 + r] = 1.0
            RA[112 + r, 4 + 3 * w + 1] = 1.0
            # chunk3 (i=2) -> pb; factors match yba rows 32..63
            if w > 0:
                RB[r, w] = 1.0
            RB[16 + r, 4 + 3 * w + 2] = 1.0
    return (
        WA0.astype(bf16),
        W1X.astype(bf16),
        BA.astype(bf16),
        RA.astype(bf16),
        RB.astype(bf16),
    )


def _pack_inputs(node_feat, pot_feat):
    """Transpose + pad + bf16-cast the node data. Returns (xT, yT) with
    xT [128, 4, PADDED]: chunk 0 = x0 features, chunks 1..3 = x1[:, :, i-1].
    yT [16, PADDED]."""
    bf16 = _bf16()
    n = node_feat.shape[0]
    xT = np.zeros((128, 4, PADDED), dtype=bf16)
    xT[:, 0, :n] = np.asarray(node_feat[:, :128].T, dtype=bf16)
    x1 = node_feat[:, 128:].reshape(n, 128, 3)
    for c in range(3):
        xT[:, 1 + c, :n] = np.asarray(x1[:, :, c].T, dtype=bf16)
    yT = np.zeros((16, PADDED), dtype=bf16)
    yT[:, :n] = np.asarray(pot_feat.T, dtype=bf16)
    return xT, yT


def build_in_maps(node_feat, pot_feat, w000, w011, w101, w110):
    node_feat = np.asarray(node_feat, dtype=np.float32)
    pot_feat = np.asarray(pot_feat, dtype=np.float32)
    WA0, W1X, BA, RA, RB = _build_mats(
        np.asarray(w000, np.float32),
        np.asarray(w011, np.float32),
        np.asarray(w101, np.float32),
        np.asarray(w110, np.float32),
    )
    xT, yT = _pack_inputs(node_feat, pot_feat)
    in_maps = []
    for i in range(N_CORES):
        sl = slice(i * PER_CORE, (i + 1) * PER_CORE)
        in_maps.append(
            {
                "xt": np.ascontiguousarray(xT[:, :, sl]),
                "yt": np.ascontiguousarray(yT[:, sl]),
                "wa0": WA0,
                "w1x": W1X,
                "ba": BA,
                "ra": RA,
                "rb": RB,
            }
        )
    return in_maps


_CACHE = {}


def build_kernel(n_nodes=PER_CORE):
    """Build + compile the per-core Bass program (n_nodes multiple of CHUNK)."""
    if n_nodes in _CACHE:
        return _CACHE[n_nodes]

    import concourse.bacc as bacc
    import concourse.tile as tile
    from concourse import mybir

    f32 = mybir.dt.float32
    bf = mybir.dt.bfloat16

    assert n_nodes % CHUNK == 0
    n_chunks = n_nodes // CHUNK
    n_batches = n_chunks * SUB_PER_CHUNK

    nc = bacc.Bacc(None, target_bir_lowering=False)
    xtd = nc.dram_tensor("xt", [128, 4, n_nodes], bf, kind="ExternalInput")
    ytd = nc.dram_tensor("yt", [16, n_nodes], bf, kind="ExternalInput")
    wa0d = nc.dram_tensor("wa0", [128, 64], bf, kind="ExternalInput")
    w1xd = nc.dram_tensor("w1x", [128, 32], bf, kind="ExternalInput")
    bad = nc.dram_tensor("ba", [16, 128], bf, kind="ExternalInput")
    rad = nc.dram_tensor("ra", [128, 16], bf, kind="ExternalInput")
    rbd = nc.dram_tensor("rb", [32, 16], bf, kind="ExternalInput")
    outd = nc.dram_tensor("out", [16, n_nodes], bf, kind="ExternalOutput")

    with tile.TileContext(nc) as tc:
        with (
            tc.tile_pool(name="consts", bufs=1) as consts,
            tc.tile_pool(name="xin", bufs=3) as xin,
            tc.tile_pool(name="yin", bufs=3) as yin,
            tc.tile_pool(name="stg", bufs=2) as stgp,
            tc.tile_pool(name="ybs", bufs=3) as ybsp,
            tc.tile_pool(name="pas", bufs=3) as pasp,
            tc.tile_pool(name="qa", bufs=3) as qap,
            tc.tile_pool(name="qb", bufs=3) as qbp,
            tc.tile_pool(name="pa", bufs=2, space="PSUM") as pap,
            tc.tile_pool(name="pb", bufs=2, space="PSUM") as pbp,
            tc.tile_pool(name="yba", bufs=2, space="PSUM") as ybap,
            tc.tile_pool(name="otq", bufs=1, space="PSUM") as otqp,
        ):
            wa0 = consts.tile([128, 64], bf, tag="wa0")
            nc.sync.dma_start(out=wa0[:], in_=wa0d[:])
            w1x = consts.tile([128, 32], bf, tag="w1x")
            nc.sync.dma_start(out=w1x[:], in_=w1xd[:])
            ba = consts.tile([16, 128], bf, tag="ba")
            nc.sync.dma_start(out=ba[:], in_=bad[:])
            ra = consts.tile([128, 16], bf, tag="ra")
            nc.sync.dma_start(out=ra[:], in_=rad[:])
            rbt = consts.tile([64, 16], bf, tag="rb")
            nc.sync.dma_start(out=rbt[32:64, :], in_=rbd[:])

            X = Y = None
            otq_cur = [None]  # current 2-batch group tile
            pending = []  # reduce states, drained at pipeline distance 2

            def emit_reduce(state):
                """Reduce one batch into its bank of a 2-bank group tile;
                after both batches of the group, one cast + one contiguous
                DMA moves the group out (halves the DVE/DMA op count).
                Each batch's R_A/R_B accumulation stays within ONE bank, so
                concurrent matmuls of adjacent batches never share a bank."""
                qa_, qb_, gg = state
                j = gg % 2
                if j == 0:
                    otq_cur[0] = otqp.tile(
                        [16, 2 * BATCH], f32, tag="otq", name=f"otq_{gg}"
                    )
                otq = otq_cur[0]
                sl = otq[:, j * BATCH : (j + 1) * BATCH]
                nc.tensor.matmul(sl, ra[:], qa_[:], start=True, stop=False)
                nc.tensor.matmul(
                    sl, rbt[32:64, :], qb_[32:64, :], start=False, stop=True
                )
                if j == 1:
                    stage = stgp.tile([16, 2 * BATCH], bf, tag="stg")
                    nc.vector.tensor_copy(stage[:], otq[:])
                    # out-DMA on the ACT HWDGE ring: the Sync ring carries
                    # the input prefetch and must never head-of-line block
                    # on a DMA that waits for compute.
                    c0 = (gg - 1) * BATCH
                    nc.scalar.dma_start(
                        out=outd[:, c0 : c0 + 2 * BATCH], in_=stage[:]
                    )

            for g in range(n_batches):
                ch, sb = divmod(g, SUB_PER_CHUNK)
                j0 = ch * CHUNK
                if sb == 0:
                    Y = yin.tile([16, CHUNK], bf, tag="y")
                    nc.sync.dma_start(out=Y[:], in_=ytd[:, j0 : j0 + CHUNK])
                    X = xin.tile([128, 4 * CHUNK], bf, tag="x")
                    nc.sync.dma_start(
                        out=X[:].rearrange("p (c m) -> p c m", c=4),
                        in_=xtd[:, :, j0 : j0 + CHUNK],
                    )

                m0 = sb * BATCH
                # broadcast y into the 128 A-rows (selector matmul)
                yba = ybap.tile([128, BATCH], f32, tag="yba")
                nc.tensor.matmul(
                    yba[:], ba[:], Y[:, m0 : m0 + BATCH], start=True, stop=True
                )
                ybs = ybsp.tile([128, BATCH], bf, tag="ybs")
                nc.scalar.copy(out=ybs[:], in_=yba[:])
                # stage 1: contract the 512 node features.
                # PE slot structure per batch (disjoint array cells overlap):
                #   [c0|c1|c2|pb] -> [R_A(g-2)|R_B(g-3)] -> Yb_A
                pa = pap.tile([128, BATCH], f32, tag="pa")
                nc.tensor.matmul(
                    pa[0:64, :],
                    wa0[:],
                    X[:, 0 * CHUNK + m0 : 0 * CHUNK + m0 + BATCH],
                    start=True,
                    stop=True,
                )
                nc.tensor.matmul(
                    pa[64:96, :],
                    w1x[:],
                    X[:, 1 * CHUNK + m0 : 1 * CHUNK + m0 + BATCH],
                    start=True,
                    stop=True,
                )
                nc.tensor.matmul(
                    pa[96:128, :],
                    w1x[:],
                    X[:, 2 * CHUNK + m0 : 2 * CHUNK + m0 + BATCH],
                    start=True,
                    stop=True,
                    tile_position=(0, 96),
                )
                pb = pbp.tile([64, BATCH], f32, tag="pb")
                nc.tensor.matmul(
                    pb[32:64, :],
                    w1x[:],
                    X[:, 3 * CHUNK + m0 : 3 * CHUNK + m0 + BATCH],
                    start=True,
                    stop=True,
                )
                pas = pasp.tile([128, BATCH], bf, tag="pas")
                nc.scalar.copy(out=pas[:], in_=pa[:])
                # reduce of batch g-2 (q-chain has 2 batches of slack)
                if len(pending) >= 2:
                    emit_reduce(pending.pop(0))
                # q = p * yb (qa on gpsimd frees the DVE for qb+output)
                qa = qap.tile([128, BATCH], bf, tag="qa")
                nc.gpsimd.tensor_mul(qa[:], pas[:], ybs[:])
                qb = qbp.tile([64, BATCH], bf, tag="qb")
                nc.vector.tensor_mul(qb[32:64, :], pb[32:64, :], ybs[32:64, :])
                pending.append((qa, qb, g))
            for state in pending:
                emit_reduce(state)
    nc.compile()
    _CACHE[n_nodes] = nc
    return nc


def kernel(node_feat, pot_feat, w000, w011, w101, w110, **extra_kwargs):
    from concourse.bass_utils import run_bass_kernel_spmd

    n = np.asarray(node_feat).shape[0]
    in_maps = build_in_maps(node_feat, pot_feat, w000, w011, w101, w110)
    nc = build_kernel(PER_CORE)
    res = run_bass_kernel_spmd(nc, in_maps, core_ids=list(range(N_CORES)))
    outT = np.concatenate(
        [np.asarray(res.results[i]["out"]) for i in range(N_CORES)], axis=1
    )
    out = outT[:, :n].T.astype(np.float32)
    return np.ascontiguousarray(out)
